# revision 6
# baseline (speedup 1.0000x reference)
"""Trainium2 Bass kernel for a 1-layer tanh RNN + ReLU + FC head.

Reference computation (jax, fp32):
    xw = einsum('sbi,hi->sbh', x, W_ih) + (b_ih + b_hh)      # [S,B,H]
    h_{t+1} = tanh(xw[t] + h_t @ W_hh.T)                      # scan over S
    y = relu(outputs) @ fc_w.T + fc_b                         # [S,B,1]
    returns (y, h_last[None])

Shapes: S=1024, B=256, IN=6, H=512.

Strategy: data-parallel over batch across 8 NeuronCores (32 batch each),
recurrence kept local per core (no collectives).

Per-core layout ("transposed h"): h^T tiles [128 part, 4*32] where
element [p, 32*K + b] = h[128*K + p, b].  Per step, 16 matmuls
(W_hh^T 128x128 tiles stationary, h^T slices moving, N=32) accumulate
z = W_hh @ h into one PSUM tile [128, 128]; DVE adds the precomputed
xw^T slice; ACT applies tanh; DVE relu-casts to bf16; 4 more matmuls
(fc block columns stationary) reduce relu(h)*fc across partitions into
an accumulating y PSUM bank.  xw^T is produced by a separate f32r GEMM
phase (K=7 with a ones-row folding in the bias) that streams through a
DRAM scratch buffer and is prefetched into an SBUF ring.

Matmul dtype: float32r (relaxed fp32, ~1.5e-4 rel err) for the
recurrence and xw GEMM; bf16 for the y head (no error accumulation).
"""

import os
from contextlib import ExitStack

import numpy as np

import concourse.bass as bass
import concourse.tile as tile
from concourse import bacc, mybir
from concourse import bass_utils

SEQ, BATCH, IN, DIM = 1024, 256, 6, 512
NCORES = 8
BL = BATCH // NCORES          # 32 local batch
KB = DIM // 128               # 4 k/j blocks
CHUNK = 16                    # recurrence steps per xw chunk (512 moving cols)
RING = 4                      # xw chunks buffered in SBUF

F32 = mybir.dt.float32
F32R = mybir.dt.float32r
BF16 = mybir.dt.bfloat16


def build(seq: int = SEQ, rep: int = 1):
    """Build the Bass program (same NEFF for all cores; SPMD over inputs).

    rep > 1 repeats the whole compute body (timing calibration only).
    """
    nchunks = seq // CHUNK
    nc = bacc.Bacc(
        "TRN2",
        target_bir_lowering=False,
        debug=False,
        enable_asserts=False,
        num_devices=NCORES,
    )

    # ---- I/O ----
    x_aug = nc.dram_tensor("x_aug", (IN + 1, seq * BL), F32R, kind="ExternalInput").ap()
    w_ih_aug = nc.dram_tensor("w_ih_aug", (IN + 1, DIM), F32R, kind="ExternalInput").ap()
    w_tiles = nc.dram_tensor("w_tiles", (128, KB * KB * 128), F32R, kind="ExternalInput").ap()
    fc_blk = nc.dram_tensor("fc_blk", (128, KB), BF16, kind="ExternalInput").ap()
    h0_t = nc.dram_tensor("h0_t", (128, KB * BL), F32R, kind="ExternalInput").ap()
    fcb_in = nc.dram_tensor("fcb", (1, 1), F32, kind="ExternalInput").ap()

    y_out = nc.dram_tensor("y_out", (1, seq * BL), F32, kind="ExternalOutput").ap()
    h_out = nc.dram_tensor("h_out", (128, KB * BL), F32R, kind="ExternalOutput").ap()

    with tile.TileContext(nc) as tc:
        with ExitStack() as ctx:
            const_pool = ctx.enter_context(tc.tile_pool(name="const", bufs=1))
            xin_pool = ctx.enter_context(tc.tile_pool(name="xin", bufs=4))
            stage_pool = ctx.enter_context(tc.tile_pool(name="stage", bufs=4))
            ring_pool = ctx.enter_context(tc.tile_pool(name="ring", bufs=RING))
            z_pool = ctx.enter_context(tc.tile_pool(name="z", bufs=2))
            h_pool = ctx.enter_context(tc.tile_pool(name="h", bufs=3))
            r_pool = ctx.enter_context(tc.tile_pool(name="relu", bufs=2))
            ysb_pool = ctx.enter_context(tc.tile_pool(name="ysb", bufs=1))
            psA_pool = ctx.enter_context(tc.tile_pool(name="psA", bufs=2, space="PSUM"))
            psz_pool = ctx.enter_context(tc.tile_pool(name="psz", bufs=2, space="PSUM"))
            psy_pool = ctx.enter_context(tc.tile_pool(name="psy", bufs=2, space="PSUM"))
            dram_pool = ctx.enter_context(tc.tile_pool(name="dram", bufs=1, space="DRAM"))

            # ---- constants into SBUF ----
            w_sb = const_pool.tile([128, KB * KB * 128], F32R)
            nc.sync.dma_start(w_sb[:], w_tiles[:])
            wih_sb = const_pool.tile([IN + 1, DIM], F32R)
            nc.sync.dma_start(wih_sb[:], w_ih_aug[:])
            fc_sb = const_pool.tile([128, KB], BF16)
            nc.sync.dma_start(fc_sb[:], fc_blk[:])
            fcb_sb = const_pool.tile([1, 1], F32)
            nc.sync.dma_start(fcb_sb[:], fcb_in[:])

            # y staging buffer in SBUF (single partition)
            y_sb = ysb_pool.tile([1, seq * BL], F32)

            # DRAM scratch for xw^T: [KB, 128, seq, BL]
            xw_dram = dram_pool.tile([KB, 128, seq, BL], F32)

            for _rep in range(rep):
                _phases(nc, tc, locals())

    nc.compile()
    return nc


def _phases(nc, tc, env):
    seq = env["seq"]
    nchunks = env["nchunks"]
    xin_pool = env["xin_pool"]
    stage_pool = env["stage_pool"]
    ring_pool = env["ring_pool"]
    z_pool = env["z_pool"]
    h_pool = env["h_pool"]
    r_pool = env["r_pool"]
    psA_pool = env["psA_pool"]
    psz_pool = env["psz_pool"]
    psy_pool = env["psy_pool"]
    w_sb = env["w_sb"]
    wih_sb = env["wih_sb"]
    fc_sb = env["fc_sb"]
    fcb_sb = env["fcb_sb"]
    y_sb = env["y_sb"]
    xw_dram = env["xw_dram"]
    x_aug = env["x_aug"]
    h0_t = env["h0_t"]
    y_out = env["y_out"]
    h_out = env["h_out"]

    if True:
            # ---- Phase A: xw^T GEMM (K=7, f32r), chunked over seq ----
            for c in range(nchunks):
                x_sb = xin_pool.tile([IN + 1, CHUNK * BL], F32R)
                nc.sync.dma_start(x_sb[:], x_aug[:, c * CHUNK * BL:(c + 1) * CHUNK * BL])
                for B in range(KB):
                    psA = psA_pool.tile([128, CHUNK * BL], F32)
                    nc.tensor.matmul(
                        psA[:],
                        wih_sb[:, B * 128:(B + 1) * 128],
                        x_sb[:],
                        start=True, stop=True,
                    )
                    stg = stage_pool.tile([128, CHUNK * BL], F32)
                    # alternate copy engines to halve the copy bottleneck
                    if B % 2 == 0:
                        nc.scalar.copy(stg[:], psA[:])
                    else:
                        nc.vector.tensor_copy(stg[:], psA[:])
                    nc.sync.dma_start(
                        xw_dram[B, :, c * CHUNK:(c + 1) * CHUNK, :],
                        stg[:],
                    )

            # ---- Phase B: recurrence ----
            h_cur = h_pool.tile([128, KB * BL], F32R)
            nc.sync.dma_start(h_cur[:], h0_t[:])

            psy = None
            ring = None
            for t in range(seq):
                c, t16 = divmod(t, CHUNK)
                if t16 == 0:
                    ring = ring_pool.tile([128, KB, CHUNK, BL], F32)
                    for B in range(KB):
                        nc.sync.dma_start(
                            ring[:, B, :, :],
                            xw_dram[B, :, c * CHUNK:(c + 1) * CHUNK, :],
                        )
                    psy = psy_pool.tile([1, CHUNK * BL], F32)

                # 16 recurrence matmuls into one PSUM tile [128, 128]
                psz = psz_pool.tile([128, KB * BL], F32)
                for B in range(KB):
                    for k in range(KB):
                        nc.tensor.matmul(
                            psz[:, B * BL:(B + 1) * BL],
                            w_sb[:, (k * KB + B) * 128:(k * KB + B + 1) * 128],
                            h_cur[:, k * BL:(k + 1) * BL],
                            start=(k == 0), stop=(k == KB - 1),
                        )

                # z = psz + xw[t]
                z_sb = z_pool.tile([128, KB * BL], F32)
                nc.vector.tensor_add(z_sb[:], psz[:], ring[:, :, t16, :])
                # h = tanh(z)
                h_nxt = h_pool.tile([128, KB * BL], F32R)
                nc.scalar.activation(
                    h_nxt[:], z_sb[:], mybir.ActivationFunctionType.Tanh,
                )
                # relu cast to bf16 for the y head
                r_sb = r_pool.tile([128, KB * BL], BF16)
                nc.vector.tensor_scalar_max(r_sb[:], h_nxt[:], 0.0)
                # y[t] = sum_p relu(h) * fc  (fc block-columns stationary)
                for k in range(KB):
                    nc.tensor.matmul(
                        psy[0:1, t16 * BL:(t16 + 1) * BL],
                        fc_sb[:, k:k + 1],
                        r_sb[:, k * BL:(k + 1) * BL],
                        start=(k == 0), stop=(k == KB - 1),
                    )
                if t16 == CHUNK - 1:
                    # move 16 accumulated y columns to SBUF, adding fc_b
                    nc.vector.tensor_scalar(
                        y_sb[0:1, c * CHUNK * BL:(c + 1) * CHUNK * BL],
                        psy[0:1, :],
                        fcb_sb[0:1, 0:1],
                        None,
                        op0=mybir.AluOpType.add,
                    )
                h_cur = h_nxt

            nc.sync.dma_start(h_out[:], h_cur[:])
            nc.sync.dma_start(y_out[:], y_sb[:])


_NC_CACHE = {}


def _get_nc(seq: int = SEQ):
    if seq not in _NC_CACHE:
        _NC_CACHE[seq] = build(seq)
    return _NC_CACHE[seq]


def prep_inputs(x, hidden, W_ih, W_hh, b_ih, b_hh, fc_w, fc_b, seq=SEQ):
    """Host-side layout prep -> list of per-core input dicts."""
    import ml_dtypes

    x = np.asarray(x, np.float32)
    hidden = np.asarray(hidden, np.float32)
    W_ih = np.asarray(W_ih, np.float32)
    W_hh = np.asarray(W_hh, np.float32)
    b_ih = np.asarray(b_ih, np.float32)
    b_hh = np.asarray(b_hh, np.float32)
    fc_w = np.asarray(fc_w, np.float32)
    fc_b = np.asarray(fc_b, np.float32)

    # x^T: [6, seq, 256]
    xT = np.ascontiguousarray(x[:seq].transpose(2, 0, 1))
    # W tiles: [p, (k*KB+B)*128 + m] = W_hh[128B+m, 128k+p]
    W4 = W_hh.reshape(KB, 128, KB, 128)                            # [B, m, k, p]
    w_tiles = np.ascontiguousarray(W4.transpose(3, 2, 0, 1).reshape(128, KB * KB * 128))
    # w_ih_aug: [7, 512]
    w_ih_aug = np.ascontiguousarray(
        np.concatenate([W_ih.T, (b_ih + b_hh)[None, :]], axis=0).astype(np.float32))
    # fc blocks: [128, KB] bf16
    fc_blk = np.ascontiguousarray(fc_w[0].reshape(KB, 128).T).astype(ml_dtypes.bfloat16)
    fcb = np.array([[fc_b[0]]], np.float32)

    in_maps = []
    for c in range(NCORES):
        sl = slice(c * BL, (c + 1) * BL)
        x_aug_c = np.concatenate(
            [xT[:, :, sl].reshape(IN, seq * BL),
             np.ones((1, seq * BL), np.float32)], axis=0)
        h0 = hidden[0, sl, :].T                                    # [512, BL]
        h0_t = np.ascontiguousarray(
            h0.reshape(KB, 128, BL).transpose(1, 0, 2).reshape(128, KB * BL))
        in_maps.append({
            "x_aug": np.ascontiguousarray(x_aug_c),
            "w_ih_aug": w_ih_aug,
            "w_tiles": w_tiles,
            "fc_blk": fc_blk,
            "h0_t": h0_t,
            "fcb": fcb,
        })
    return in_maps


def postprocess(results, seq=SEQ):
    y_full = np.zeros((seq, BATCH, 1), np.float32)
    h_full = np.zeros((1, BATCH, DIM), np.float32)
    for c, res in enumerate(results):
        sl = slice(c * BL, (c + 1) * BL)
        y_full[:, sl, 0] = res["y_out"][0].reshape(seq, BL)
        h_np = res["h_out"]                                        # [128, KB*BL]
        h_r = h_np.reshape(128, KB, BL).transpose(1, 0, 2).reshape(DIM, BL)
        h_full[0, sl, :] = h_r.T
    return y_full, h_full


def run(inputs: dict, seq=SEQ, trace=False, tmpdir=None):
    nc = _get_nc(seq)
    in_maps = prep_inputs(**inputs, seq=seq)
    res = bass_utils.run_bass_kernel_spmd(
        nc, in_maps, core_ids=list(range(NCORES)), trace=trace, tmpdir=tmpdir)
    y, h = postprocess(res.results, seq=seq)
    return (y, h), res


def kernel(**inputs):
    (y, h), _ = run(inputs, seq=SEQ)
    return y, h


if __name__ == "__main__":
    import time
    t0 = time.time()
    nc = build(int(os.environ.get("RNN_SEQ", "64")))
    print(f"build+compile took {time.time() - t0:.1f}s")
